# revision 1
# baseline (speedup 1.0000x reference)
"""GCN (3-layer) Bass kernel for 8 TRN2 NeuronCores, SPMD.

Math: out = A_hat @ relu(A_hat @ relu(A_hat @ X W1 + b1) W2 + b2) W3 + b3
where A_hat = D^-1/2 (A + I) D^-1/2 (in-degree over col of edge_index + self loop).

Factorization (per-node scaling absorbs the symmetric norm, a = dinv > 0):
  Xs = a * X                      (host-side for layer 1)
  H_sent = Xs @ W = a * (X @ W)   (GEMM epilogue is a pure f32->bf16 cast)
  Agg[c] = sum_{e: dst=c, src!=c} H_sent[src] + H_sent[c]    (self loop = PE
           identity matmul, cross edges = dma_gather + one-hot S matmuls)
  Xs_next = a * relu(a * Agg + b) = relu(a^2 * Agg + a*b)    (one ACT op,
           bias enters via a per-tile "fake block": lhsT row0 = sqrt(deg),
           rhs row0 = b, so the a^2 scale turns it into a*b)
  out (layer 3) = a * Agg + b     (ACT Copy, scale = a)

Layout per core (core owns S = N/8 contiguous dst nodes, NT = ceil(S/128)
tiles of 128):
  - GEMM on local shard -> bounce DRAM -> 2 AllGathers (top/bottom
    half-shards) -> two gather tables of TBL = N/2 rows each (so int16
    indices stay in range).
  - Cross-edge messages sorted by (src-half, dst_tile, src); each (half,tile)
    run padded to a multiple of 128 with dummy slots (idx=0, dstloc=-1).
    Block counts = max over the 8 cores (single SPMD instruction stream).
  - dma_gather (4 SWDGE queues round-robin -> ~2.5x descriptor-gen
    parallelism) pulls 256B bf16 rows message-major:
    msg[p, c, :] = row of message c*128+p.
  - S built 8 blocks per DVE op: S[p, k, j] = (dstloc[p, k] == j) via
    tensor_tensor is_equal against an iota row broadcast.
  - PE: psum[dst, feat] += S_k^T @ msg_k, one PSUM bank = 4 dst tiles.
"""

from contextlib import ExitStack

import numpy as np
import ml_dtypes

import concourse.bacc as bacc
import concourse.bass as bass
import concourse.mybir as mybir
from concourse.tile import TileContext
from concourse import library_config

BF16 = mybir.dt.bfloat16
F32 = mybir.dt.float32
I16 = mybir.dt.int16
P = 128
SBK = 16         # S-build batch (blocks per DVE op)
TG = 4           # dst tiles per PSUM bank group


def preprocess(edge_index, n_nodes, n_cores=8, call_size=1792):
    """Host-side index preprocessing. Returns (sched, per_core_data, dinv)."""
    src = np.asarray(edge_index[0], dtype=np.int64)
    dst = np.asarray(edge_index[1], dtype=np.int64)
    deg = (np.bincount(dst, minlength=n_nodes) + 1).astype(np.float32)
    dinv = 1.0 / np.sqrt(deg)
    sqd = np.sqrt(deg)

    S = n_nodes // n_cores
    assert S * n_cores == n_nodes and S % 2 == 0
    HS = S // 2
    TBL = HS * n_cores
    NT = (S + P - 1) // P

    core_of = dst // S
    shard = src // S
    sloc = src % S
    half = (sloc >= HS).astype(np.int64)
    trow = shard * HS + (sloc - half * HS)
    dloc = dst % S
    dtile = dloc // P
    dlane = dloc % P

    counts = np.zeros((n_cores, 2, NT), dtype=np.int64)
    for c in range(n_cores):
        m = core_of == c
        np.add.at(counts[c], (half[m], dtile[m]), 1)
    B = np.ceil(counts / P).max(axis=0).astype(np.int64)  # [2, NT]

    per_core = []
    for c in range(n_cores):
        m = core_of == c
        h_c, t_c, r_c, l_c = half[m], dtile[m], trow[m], dlane[m]
        idx_stream = []
        dl_stream = []
        for hh in range(2):
            for tt in range(NT):
                sel = (h_c == hh) & (t_c == tt)
                rows = r_c[sel]
                lanes = l_c[sel]
                order = np.argsort(rows, kind="stable")  # DMA locality
                rows, lanes = rows[order], lanes[order]
                pad = B[hh, tt] * P - len(rows)
                assert pad >= 0
                idx_stream.append(rows)
                idx_stream.append(np.zeros(pad, dtype=np.int64))
                dl_stream.append(lanes)
                dl_stream.append(np.full(pad, -1, dtype=np.int64))
        idx_stream = np.concatenate(idx_stream)
        dl_stream = np.concatenate(dl_stream)
        LT = len(idx_stream)
        assert LT == B.sum() * P
        idxw = idx_stream.astype(np.int16).reshape(LT // 16, 16).T
        idxw = np.tile(idxw, (8, 1))
        dl = dl_stream.reshape(LT // P, P).T.astype(np.float32)
        dvc = np.zeros((P, NT), dtype=np.float32)
        dv2 = np.zeros((P, NT), dtype=np.float32)
        sfk = np.zeros((P, NT * P), dtype=np.float32)
        ids = np.arange(S)
        dvc[ids % P, ids // P] = dinv[c * S + ids]
        dv2[ids % P, ids // P] = dinv[c * S + ids] ** 2
        sfk[0, ids] = sqd[c * S + ids]
        per_core.append(dict(idxw=idxw, dstloc=dl, dinvc=dvc, dinv2c=dv2,
                             sfake=sfk.astype(ml_dtypes.bfloat16)))

    L0 = int(B[0].sum()) * P
    L1 = int(B[1].sum()) * P
    calls = []  # (phase, start_msg, n_msgs)
    for hh, (lo, ln) in enumerate(((0, L0), (L0, L1))):
        off = 0
        while off < ln:
            n = min(call_size, ln - off)
            calls.append((hh, lo + off, n))
            off += n

    sched = dict(n_nodes=n_nodes, n_cores=n_cores, S=S, HS=HS, TBL=TBL, NT=NT,
                 B=B, L0=L0, L1=L1, calls=calls, call_size=call_size)
    return sched, per_core, dinv


def build_nc(sched):
    """Build the SPMD Bass graph (identical for all 8 cores)."""
    S, HS, TBL, NT = sched["S"], sched["HS"], sched["TBL"], sched["NT"]
    B, calls = sched["B"], sched["calls"]
    n_cores = sched["n_cores"]
    call_size = sched["call_size"]
    NB = int(B.sum())
    LT = NB * P
    core_ids = list(range(n_cores))
    n_full = S // P
    rem = S - n_full * P

    nc = bacc.Bacc("TRN2", target_bir_lowering=False, num_devices=n_cores,
                   num_swdge_queues=4)

    x_in = nc.dram_tensor("x", [NT * P, P], F32, kind="ExternalInput")
    w_in = [nc.dram_tensor(f"w{i+1}", [P, P], BF16, kind="ExternalInput")
            for i in range(3)]
    bfake_in = nc.dram_tensor("bfake", [P, 3, P], BF16, kind="ExternalInput")
    sfake_in = nc.dram_tensor("sfake", [P, NT * P], BF16, kind="ExternalInput")
    dinv_in = nc.dram_tensor("dinvc", [P, NT], F32, kind="ExternalInput")
    dinv2_in = nc.dram_tensor("dinv2c", [P, NT], F32, kind="ExternalInput")
    ident_in = nc.dram_tensor("identb", [P, P], BF16, kind="ExternalInput")
    iota_in = nc.dram_tensor("iotab", [P, SBK * P], BF16, kind="ExternalInput")
    idxw_in = nc.dram_tensor("idxw", [P, LT // 16], I16, kind="ExternalInput")
    dstloc_in = nc.dram_tensor("dstloc", [P, NB], F32, kind="ExternalInput")
    out_ext = nc.dram_tensor("out", [S, 64], F32, kind="ExternalOutput")

    with TileContext(nc) as tc, ExitStack() as ex:
        const = ex.enter_context(tc.tile_pool(name="const", bufs=1))
        dram = ex.enter_context(tc.tile_pool(name="dram", bufs=1, space="DRAM"))
        sb = ex.enter_context(tc.tile_pool(name="sb", bufs=2))
        msgp = ex.enter_context(tc.tile_pool(name="msgp", bufs=10))
        spool = ex.enter_context(tc.tile_pool(name="spool", bufs=4))
        xtp = ex.enter_context(tc.tile_pool(name="xtp", bufs=2))
        accp = ex.enter_context(tc.tile_pool(name="accp", bufs=1))
        ps_agg = ex.enter_context(tc.tile_pool(name="ps_agg", bufs=3, space="PSUM"))
        ps_gemm = ex.enter_context(tc.tile_pool(name="ps_gemm", bufs=2, space="PSUM"))
        ps_tr = ex.enter_context(tc.tile_pool(name="ps_tr", bufs=2, space="PSUM"))

        nc.gpsimd.load_library(library_config.mlp)

        def load_const(name, src_ap, shape, dtype):
            t = const.tile(shape, dtype, name=name)
            nc.sync.dma_start(t[:], src_ap)
            return t

        w_sb = [load_const(f"w{i}", w_in[i][:], [P, P], BF16) for i in range(3)]
        bfake = load_const("bfake", bfake_in[:], [P, 3, P], BF16)
        sfake = load_const("sfake", sfake_in[:], [P, NT * P], BF16)
        dinvc = load_const("dinvc", dinv_in[:], [P, NT], F32)
        dinv2c = load_const("dinv2c", dinv2_in[:], [P, NT], F32)
        identb = load_const("identb", ident_in[:], [P, P], BF16)
        iotab = load_const("iotab", iota_in[:], [P, SBK * P], BF16)
        idxw = load_const("idxw", idxw_in[:], [P, LT // 16], I16)
        dstloc = load_const("dstloc", dstloc_in[:], [P, NB], F32)

        x_prev = None  # SBUF [P, NT, P] bf16 = a*X for layers 2,3

        for layer in range(3):
            # ---- GEMM: h_sent = (a*X) @ W, pure-cast epilogue, 4-tile groups
            h_sent = sb.tile([P, NT, P], BF16, name="h_sent")
            for g in range(0, NT, TG):
                gsz = min(TG, NT - g)
                if layer == 0:
                    xf = sb.tile([P, TG, P], F32, name="xf")
                    nc.sync.dma_start(
                        xf[:, :gsz, :],
                        x_in[g * P:(g + gsz) * P, :].rearrange(
                            "(t p) f -> p t f", p=P))
                    xb = sb.tile([P, TG, P], BF16, name="xb")
                    nc.vector.tensor_copy(xb[:, :gsz, :], xf[:, :gsz, :])
                g_ps = ps_gemm.tile([P, TG, P], F32, space="PSUM", name="g_ps")
                for j in range(gsz):
                    t = g + j
                    xbj = xb[:, j, :] if layer == 0 else x_prev[:, t, :]
                    tr_ps = ps_tr.tile([P, P], BF16, space="PSUM", name="tr_ps")
                    nc.tensor.transpose(out=tr_ps[:], in_=xbj, identity=identb[:])
                    xt = xtp.tile([P, P], BF16, name="xt")
                    nc.vector.tensor_copy(xt[:], tr_ps[:])
                    nc.tensor.matmul(out=g_ps[:, j, :], lhsT=xt[:],
                                     rhs=w_sb[layer][:], start=True, stop=True)
                nc.vector.tensor_copy(h_sent[:, g:g + gsz, :], g_ps[:, :gsz, :])

            # ---- h_sent -> bounce DRAM -> two AllGathers ----
            bounce = dram.tile([S, P], BF16, name="bounce")

            def dma_rows(r0, r1):
                """DMA h_sent node rows [r0, r1) into bounce (tile-aligned
                middle as one big DMA, ragged edges separately)."""
                while r0 < r1:
                    t0, l0 = divmod(r0, P)
                    if l0 == 0 and r1 - r0 >= P:
                        tn = (r1 - r0) // P
                        nc.sync.dma_start(
                            bounce[r0:r0 + tn * P, :].rearrange(
                                "(t p) f -> p t f", p=P),
                            h_sent[:, t0:t0 + tn, :])
                        r0 += tn * P
                    else:
                        l1 = min(P, l0 + (r1 - r0))
                        nc.sync.dma_start(
                            bounce[r0:r0 + (l1 - l0), :].rearrange(
                                "(t p) f -> p t f", t=1),
                            h_sent[l0:l1, t0:t0 + 1, :])
                        r0 += l1 - l0

            dma_rows(0, HS)
            dma_rows(HS, S)
            tables = []
            for hh in range(2):
                tbl = dram.tile([TBL, P], BF16, addr_space="Shared",
                                name=f"tbl{hh}")
                nc.gpsimd.collective_compute(
                    "AllGather", mybir.AluOpType.bypass,
                    replica_groups=[core_ids],
                    ins=[bounce[hh * HS:(hh + 1) * HS, :]],
                    outs=[tbl[:]])
                tables.append(tbl)

            # ---- gather calls (4 SWDGE queues, round robin) ----
            msg_tiles = []
            for ci, (hh, start, n) in enumerate(calls):
                mt = msgp.tile([P, call_size // P, P], BF16, name="mt")
                nc.gpsimd.dma_gather(
                    mt[:, 0:n // P, :], tables[hh][:],
                    idxw[:, start // 16:(start + n) // 16],
                    n, n, P, queue_num=ci % 4)
                msg_tiles.append((start, n, mt))

            def msg_ap(ms):
                for (cs, cn, mt) in msg_tiles:
                    if cs <= ms < cs + cn:
                        return mt[:, (ms - cs) // P, :]
                raise AssertionError

            # ---- batched S builds (SBK blocks per DVE op) ----
            s_tiles = []  # block index -> (tile, slot)
            for b0 in range(0, NB, SBK):
                k = min(SBK, NB - b0)
                st = spool.tile([P, SBK, P], BF16, name="st")
                nc.vector.tensor_tensor(
                    out=st[:, :k, :],
                    in0=iotab[:, 0:k * P].rearrange("p (k j) -> p k j", k=k),
                    in1=dstloc[:, b0:b0 + k].to_broadcast([P, k, P]),
                    op=mybir.AluOpType.is_equal)
                for j in range(k):
                    s_tiles.append((st, j))

            # ---- segment sum: per phase, 4-tile PSUM groups -> acc ----
            acc = accp.tile([P, NT, P], F32, name="acc")
            gb = 0
            boff = 0
            for hh in range(2):
                for g in range(0, NT, TG):
                    gsz = min(TG, NT - g)
                    a_ps = ps_agg.tile([P, TG, P], F32, space="PSUM", name="a_ps")
                    have = []  # slices written this phase
                    for j in range(gsz):
                        t = g + j
                        nb = int(B[hh, t])
                        if hh == 0:
                            nc.tensor.matmul(
                                out=a_ps[:, j, :],
                                lhsT=sfake[:, t * P:(t + 1) * P],
                                rhs=bfake[:, layer, :], start=True, stop=False)
                            nc.tensor.matmul(
                                out=a_ps[:, j, :], lhsT=identb[:],
                                rhs=h_sent[:, t, :], start=False, stop=(nb == 0))
                        elif nb == 0:
                            continue
                        have.append(j)
                        for i in range(nb):
                            st, slot = s_tiles[gb]
                            nc.tensor.matmul(
                                out=a_ps[:, j, :], lhsT=st[:, slot, :],
                                rhs=msg_ap(boff),
                                start=(hh == 1 and i == 0), stop=(i == nb - 1))
                            gb += 1
                            boff += P
                    if hh == 0:
                        nc.vector.tensor_copy(acc[:, g:g + gsz, :], a_ps[:, :gsz, :])
                    elif len(have) == gsz:
                        nc.vector.tensor_tensor(
                            out=acc[:, g:g + gsz, :], in0=acc[:, g:g + gsz, :],
                            in1=a_ps[:, :gsz, :], op=mybir.AluOpType.add)
                    else:
                        for j in have:
                            nc.vector.tensor_tensor(
                                out=acc[:, g + j, :], in0=acc[:, g + j, :],
                                in1=a_ps[:, j, :], op=mybir.AluOpType.add)

            # ---- epilogue ----
            if layer < 2:
                x_prev = sb.tile([P, NT, P], BF16, name="x_next")
                for t in range(NT):
                    nc.scalar.activation(
                        out=x_prev[:, t, :], in_=acc[:, t, :],
                        func=mybir.ActivationFunctionType.Relu,
                        scale=dinv2c[:, t:t + 1])
            else:
                out_sb = sb.tile([P, NT, 64], F32, name="out_sb")
                for t in range(NT):
                    nc.scalar.activation(
                        out=out_sb[:, t, :], in_=acc[:, t, :64],
                        func=mybir.ActivationFunctionType.Copy,
                        scale=dinvc[:, t:t + 1])
                if n_full:
                    nc.sync.dma_start(
                        out_ext[0:n_full * P, :].rearrange("(t p) f -> p t f", p=P),
                        out_sb[:, 0:n_full, :])
                if rem:
                    nc.sync.dma_start(
                        out_ext[n_full * P:S, :].rearrange("(t p) f -> p t f", t=1),
                        out_sb[0:rem, n_full:NT, :])

    nc.compile()
    return nc


def make_in_maps(x, W1, b1, W2, b2, W3, b3, sched, per_core, dinv):
    """Build per-core input dicts (x pre-scaled by dinv)."""
    S, NT = sched["S"], sched["NT"]
    n_cores = sched["n_cores"]
    bf = ml_dtypes.bfloat16
    w1 = np.asarray(W1, np.float32).astype(bf)
    w2 = np.asarray(W2, np.float32).astype(bf)
    w3 = np.zeros((P, P), np.float32)
    w3[:, :64] = np.asarray(W3, np.float32)
    w3 = w3.astype(bf)
    bfake = np.zeros((P, 3, P), np.float32)
    bfake[0, 0, :] = np.asarray(b1, np.float32)
    bfake[0, 1, :] = np.asarray(b2, np.float32)
    bfake[0, 2, :64] = np.asarray(b3, np.float32)
    bfake = bfake.astype(bf)
    identb = np.eye(P, dtype=np.float32).astype(bf)
    iotab = np.tile(np.arange(P, dtype=np.float32), (P, SBK)).astype(bf)
    xs = np.asarray(x, np.float32) * np.asarray(dinv)[:, None]

    in_maps = []
    for c in range(n_cores):
        d = per_core[c]
        xp = np.zeros((NT * P, P), np.float32)
        xp[:S] = xs[c * S:(c + 1) * S]
        in_maps.append({
            "x": xp,
            "w1": w1, "w2": w2, "w3": w3,
            "bfake": bfake,
            "sfake": np.ascontiguousarray(d["sfake"]),
            "dinvc": np.ascontiguousarray(d["dinvc"]),
            "dinv2c": np.ascontiguousarray(d["dinv2c"]),
            "identb": identb, "iotab": iotab,
            "idxw": np.ascontiguousarray(d["idxw"]),
            "dstloc": np.ascontiguousarray(d["dstloc"]),
        })
    return in_maps


# ---------------------------------------------------------------------------
# Entry point: full inputs in, full output out.  Hardcoded problem shapes.
# ---------------------------------------------------------------------------
N_NODES = 50000
N_CORES = 8
CALL_SIZE = 1024


def _run(inputs, trace=False):
    from concourse.bass_utils import run_bass_kernel_spmd

    x = np.asarray(inputs["x"], np.float32)
    edge_index = np.asarray(inputs["edge_index"])
    sched, per_core, dinv = preprocess(edge_index, N_NODES, N_CORES, CALL_SIZE)
    nc = build_nc(sched)
    in_maps = make_in_maps(x, inputs["W1"], inputs["b1"], inputs["W2"],
                           inputs["b2"], inputs["W3"], inputs["b3"],
                           sched, per_core, dinv)
    res = run_bass_kernel_spmd(nc, in_maps, list(range(N_CORES)), trace=trace)
    out = np.concatenate([np.asarray(res.results[c]["out"])
                          for c in range(N_CORES)], axis=0)
    return out.astype(np.float32), res


def kernel(x, edge_index, W1, b1, W2, b2, W3, b3):
    out, _ = _run(dict(x=x, edge_index=edge_index, W1=W1, b1=b1, W2=W2,
                       b2=b2, W3=W3, b3=b3), trace=False)
    return out

